# revision 25
# baseline (speedup 1.0000x reference)
"""Trainium2 Bass kernel for a cross-attention transformer block.

Sharding: 8 cores = 4 batches x 2 query-row halves (pure data parallel,
no collectives). Each core computes the full block for its 1024 query
tokens, duplicating only the K/V projections for the other half's rows.

v2 (fp8): all large GEMMs run fp8e4m3 with DoubleRow perf mode
(2 k-tiles contracted per matmul); attention probabilities are fp8e5m2
(scaled by 1/4, exact-cancelling in the softmax ratio) so the o-matmul
weight loads run at FWL fp8 rate; q/k LN mean-subtraction is folded into
host-side centered projection weights (LN(q - c) == LN(q)); the LN+rope
chain runs bf16 after a single fp32 PSUM read; softmax denominators are
normalized with two batched broadcast multiplies per head-pair chunk.

On-device layout convention:
  - residual stream kept feature-major [C(part), tokens(free)] in fp32
  - matmuls fp8 (fp32 PSUM accumulation)
  - attention q/k LN + RoPE done token-major [tokens(part), 64(free)],
    then PE-transposed to feature-major per head
  - softmax without max subtraction (qk-layernorm bounds |s*scale| < 9.1
    incl. fp8 margin; p = exp(s*scale)/4 fits fp8e5m2 with headroom)
"""

import os
import sys
import contextlib

for _p in ("/opt/trn_rl_repo",):
    if os.path.isdir(_p) and _p not in sys.path:
        sys.path.append(_p)

import numpy as np
import ml_dtypes

import concourse.bass as bass
import concourse.mybir as mybir
import concourse.tile as tile
from concourse import bacc
from concourse.bass_utils import run_bass_kernel_spmd
from concourse.masks import make_identity

BF16 = mybir.dt.bfloat16
F32 = mybir.dt.float32
F8E4 = mybir.dt.float8e4
F8E5 = mybir.dt.float8e5
I8 = mybir.dt.int8
AF = mybir.ActivationFunctionType
OP = mybir.AluOpType
DRM = mybir.MatmulPerfMode.DoubleRow

B, N, M, C, H = 4, 2048, 1024, 1024, 16
HD = C // H            # 64
HID = 4 * C            # 4096
SCALE = 1.0 / np.sqrt(HD)
EPS = 1e-6
NQ = N // 2            # own query tokens per core (1024)
NT = N // 128          # 16 token tiles of full seq
NTQ = NQ // 128        # 8 own token tiles
MT = M // 128          # 8 ctx token tiles
CT = C // 128          # 8 feature tiles
JT = HID // 128        # 32 hidden tiles

# exp -> fp8e5m2, probabilities scaled by 2^-EXP_SHIFT (cancels in ratio)
EXP_SHIFT = 2
FE_A5 = float(4.0 * SCALE / np.log(2.0))
FE_B5 = float(4.0 * (15 - EXP_SHIFT) - 0.172 + 0.5)

_CACHE = {}


def _build_program(flags):
    """Build the single-core Tile program. `flags` controls optional beta
    and bias paths (all-zero ones are skipped)."""
    nc = bacc.Bacc("TRN2", target_bir_lowering=False, debug=False)

    def din(name, shape, dt):
        return nc.dram_tensor(name, list(shape), dt, kind="ExternalInput").ap()

    # --- DRAM inputs (per core) ---
    XT = din("xT", (C, N), F8E4)                 # x[b].T fp8, own rows first
    XOWN = din("x_own", (C, NQ), F32)            # fp32 residual basis
    CTXT = din("ctxT", (C, M), F8E4)
    WQKV = din("wqkv", (C, 3 * C), F8E4)         # q/k cols centered per head
    SAWO = din("sa_wo", (C, C), F8E4)
    CAWQ = din("ca_wq", (C, C), F8E4)
    CAWK = din("ca_wk", (C, C), F8E4)
    CAWV = din("ca_wv", (C, C), F8E4)
    CAWO = din("ca_wo", (C, C), F8E4)
    W1G = din("w1g", (C, HID), F8E4)
    W1X = din("w1x", (C, HID), F8E4)
    W2 = din("w2", (HID, C), F8E4)
    COSQ_SA = din("cosq_sa", (NQ, HD), BF16)     # cos*g*8 tables
    WQ_SA = din("wq_sa", (NQ, HD), BF16)
    COSK_SA = din("cosk_sa", (N, HD), BF16)
    WK_SA = din("wk_sa", (N, HD), BF16)
    COSQ_CA = din("cosq_ca", (NQ, HD), BF16)
    WQ_CA = din("wq_ca", (NQ, HD), BF16)
    CAKG = din("cakg", (128, HD), BF16)          # gamma*8 row (bc over heads)
    B1GF = din("b1g_f", (128, JT), F32)
    B1XF = din("b1x_f", (128, JT), F32)
    LS0 = din("ls0_f", (128, CT), F32)
    LS1 = din("ls1_f", (128, CT), F32)
    LS2 = din("ls2_f", (128, CT), F32)
    GATE = din("gate_f", (128, CT), F32)
    SABO = din("sa_bo_row", (1, C), BF16) if flags["sa_bo"] else None
    CABO = din("ca_bo_row", (1, C), BF16) if flags["ca_bo"] else None
    B2R = din("b2_row", (1, C), BF16) if flags["b2"] else None
    BQ_SA = din("bq_sa", (NQ, HD), F32) if flags["bq_sa"] else None
    BK_SA = din("bk_sa", (N, HD), F32) if flags["bk_sa"] else None
    BQ_CA = din("bq_ca", (NQ, HD), F32) if flags["bq_ca"] else None
    CAKB = din("cakb", (128, HD), F32) if flags["cakb"] else None

    Y = nc.dram_tensor("y", [C, NQ], F32, kind="ExternalOutput").ap()

    with tile.TileContext(nc) as tc:
        with contextlib.ExitStack() as top:
            consts = top.enter_context(tc.tile_pool(name="consts", bufs=1))
            residf = top.enter_context(tc.tile_pool(name="residf", bufs=1))
            residq = top.enter_context(tc.tile_pool(name="residq", bufs=1))

            # ---- constants ----
            ident = consts.tile([128, 128], BF16)
            make_identity(nc, ident[:])
            eps_t = consts.tile([128, 1], F32)
            nc.vector.memset(eps_t[:], EPS * HD)     # folded: sqrt(ssq+64eps)
            mln4 = consts.tile([128, 1], F32)
            nc.vector.memset(mln4[:], float(-np.log(2.0 ** EXP_SHIFT)))
            ones_row = consts.tile([1, 512], BF16)
            nc.vector.memset(ones_row[:], 1.0)

            def load_const(ap_in, shape, dt, tag):
                t = consts.tile(list(shape), dt, tag=tag)
                nc.sync.dma_start(t[:], ap_in)
                return t

            ls0 = load_const(LS0[:], (128, CT), F32, "ls0")
            ls1 = load_const(LS1[:], (128, CT), F32, "ls1")
            ls2 = load_const(LS2[:], (128, CT), F32, "ls2")
            gate = load_const(GATE[:], (128, CT), F32, "gate")
            b1g = load_const(B1GF[:], (128, JT), F32, "b1g")
            b1x = load_const(B1XF[:], (128, JT), F32, "b1x")
            cakg = load_const(CAKG[:], (128, HD), BF16, "cakg")
            sabo = load_const(SABO[:], (1, C), BF16, "sabo") if SABO is not None else None
            cabo = load_const(CABO[:], (1, C), BF16, "cabo") if CABO is not None else None
            b2r = load_const(B2R[:], (1, C), BF16, "b2r") if B2R is not None else None
            cakb = (load_const(CAKB[:], (128, HD), F32, "cakb")
                    if CAKB is not None else None)

            # rope tables: [128, ntiles, 64]
            def load_tab(ap_in, ntile, tag, dt=BF16):
                t = consts.tile([128, ntile, HD], dt, tag=tag)
                nc.gpsimd.dma_start(t[:], ap_in.rearrange("(i p) d -> p i d", p=128))
                return t

            cosq_sa = load_tab(COSQ_SA[:], NTQ, "cosqsa")
            wq_sa = load_tab(WQ_SA[:], NTQ, "wqsa")
            cosk_sa = load_tab(COSK_SA[:], NT, "cosksa")
            wk_sa = load_tab(WK_SA[:], NT, "wksa")
            cosq_ca = load_tab(COSQ_CA[:], NTQ, "cosqca")
            wq_ca = load_tab(WQ_CA[:], NTQ, "wqca")
            bq_sa = load_tab(BQ_SA[:], NTQ, "bqsa", F32) if BQ_SA is not None else None
            bk_sa = load_tab(BK_SA[:], NT, "bksa", F32) if BK_SA is not None else None
            bq_ca = load_tab(BQ_CA[:], NTQ, "bqca", F32) if BQ_CA is not None else None

            # lt1 = ls1 * tanh(gate)
            th = consts.tile([128, CT], F32)
            nc.scalar.activation(out=th[:], in_=gate[:], func=AF.Tanh)
            lt1 = consts.tile([128, CT], F32)
            nc.vector.tensor_mul(lt1[:], ls1[:], th[:])

            # ============ helpers ============
            def _bc_heads(ap2):
                """[128, 64] table -> [128, 8, 64] broadcast view (step-0)."""
                return bass.AP(tensor=ap2.tensor, offset=ap2.offset,
                               ap=[list(ap2.ap[0]), [0, 8], list(ap2.ap[1])])

            def _bc_inner(ap2, n):
                """[128, k] per-head scalars -> [128, k, n] broadcast view."""
                return bass.AP(tensor=ap2.tensor, offset=ap2.offset,
                               ap=[list(ap2.ap[0]), list(ap2.ap[1]), [0, n]])

            def _swap512(ap2):
                """[128, 512] -> pair-swapped view [128, 256, 2]."""
                return bass.AP(tensor=ap2.tensor, offset=ap2.offset + 1,
                               ap=[list(ap2.ap[0]), [2, 256], [-1, 2]])

            def ln_rope_chunk(ps, work, trps, heads0, cos_t, w_t, b_t,
                              dest, dest_col, tabi, alt=0):
                """LN + RoPE on a [128, 512] psum chunk (8 heads; mean is
                pre-folded into centered weights), writing paired-transposed
                fp8 into dest[:, jp, dest_col:+128].
                cos_t None => no rope (plain gamma via cakg)."""
                ssq = work.tile([128, 8], F32, tag="ssq")
                sq = work.tile([128, 512], BF16, tag="wsq")
                nc.scalar.square(sq[:], ps[:])
                nc.vector.reduce_sum(out=ssq[:], in_=sq[:].rearrange(
                    "p (h d) -> p h d", d=HD), axis=mybir.AxisListType.X)
                std8 = work.tile([128, 8], F32, tag="std8")
                nc.scalar.activation(out=std8[:], in_=ssq[:], func=AF.Sqrt,
                                     bias=eps_t[:])
                r8 = work.tile([128, 8], F32, tag="r8")
                nc.vector.reciprocal(r8[:], std8[:])
                # t0 = ps * (1/std8)  (true qn/8; the 8 is folded in tables)
                t0 = work.tile([128, 512], BF16, tag="wt0")
                nc.vector.tensor_mul(t0[:].rearrange("p (h d) -> p h d", d=HD),
                                     ps[:].rearrange("p (h d) -> p h d", d=HD),
                                     _bc_inner(r8[:], HD))
                qr = work.tile([128, 512], BF16, tag="qr")
                if cos_t is not None:
                    t1 = work.tile([128, 512], BF16, tag="wt1")
                    nc.vector.tensor_mul(t1[:].rearrange("p (h d) -> p h d", d=HD),
                                         t0[:].rearrange("p (h d) -> p h d", d=HD),
                                         _bc_heads(cos_t[:, tabi, :]))
                    t2 = work.tile([128, 512], BF16, tag="wt2")
                    nc.vector.tensor_mul(t2[:].rearrange("p (h d) -> p h d", d=HD),
                                         t0[:].rearrange("p (h d) -> p h d", d=HD),
                                         _bc_heads(w_t[:, tabi, :]))
                    if b_t is None:
                        nc.vector.tensor_add(qr[:].rearrange("p (a b) -> p a b", b=2),
                                             t1[:].rearrange("p (a b) -> p a b", b=2),
                                             _swap512(t2[:]))
                    else:
                        t3 = work.tile([128, 512], BF16, tag="wt3")
                        nc.vector.tensor_add(t3[:].rearrange("p (a b) -> p a b", b=2),
                                             t1[:].rearrange("p (a b) -> p a b", b=2),
                                             _swap512(t2[:]))
                        nc.vector.tensor_add(qr[:].rearrange(
                            "p (h d) -> p h d", d=HD), t3[:].rearrange(
                            "p (h d) -> p h d", d=HD), _bc_heads(b_t[:, tabi, :]))
                else:
                    # CA k: gamma (+ beta) broadcast over heads
                    if cakb is None:
                        nc.vector.tensor_mul(qr[:].rearrange("p (h d) -> p h d", d=HD),
                                             t0[:].rearrange("p (h d) -> p h d", d=HD),
                                             _bc_heads(cakg[:]))
                    else:
                        t3 = work.tile([128, 512], BF16, tag="wt3")
                        nc.vector.tensor_mul(t3[:].rearrange("p (h d) -> p h d", d=HD),
                                             t0[:].rearrange("p (h d) -> p h d", d=HD),
                                             _bc_heads(cakg[:]))
                        nc.vector.tensor_add(qr[:].rearrange("p (h d) -> p h d", d=HD),
                                             t3[:].rearrange("p (h d) -> p h d", d=HD),
                                             _bc_heads(cakb[:]))
                # paired transposes: [128 t, 128 (dA|dB)] -> [128 d-pair, 128 t]
                trt = trps.tile([128, 512], BF16, tag="trq")
                for jp2 in range(4):
                    nc.tensor.transpose(trt[:, jp2 * 128:(jp2 + 1) * 128],
                                        qr[:, jp2 * 128:(jp2 + 1) * 128],
                                        ident[:])
                jp0 = heads0 // 2
                nc.any.tensor_copy(
                    dest[:, jp0:jp0 + 4, dest_col:dest_col + 128],
                    trt[:].rearrange("p (j t) -> p j t", t=128))

            def attention(kf_t, v_t, qf_t, of_t, ktiles):
                """Pair-fused attention: for each head pair, s^T = k^T q into a
                2-bank psum, p = exp(scale s)/4 in fp8e5m2 alternating ACT /
                DVE-int8-fast-exp2, o = p^T v_aug with ones-column
                denominators, batched normalize, pair-transpose to of_t."""
                with tc.tile_pool(name="att_ps", bufs=2, space="PSUM") as ps_s, \
                     tc.tile_pool(name="att_po", bufs=3, space="PSUM") as ps_o, \
                     tc.tile_pool(name="att_tr", bufs=1, space="PSUM") as ps_tr, \
                     tc.tile_pool(name="att_wk", bufs=6) as wk:
                    for tqc in range(2):
                        for jp in range(CT):
                            o_A = ps_o.tile([128, 4, 128], F32, tag="ops")
                            o_B = ps_o.tile([128, 4, 128], F32, tag="ops")

                            def emit_o(pv, tk):
                                for q8 in range(4):
                                    nc.tensor.matmul(
                                        o_A[:, q8, 0:65],
                                        pv[:, 0, q8 * 128:(q8 + 1) * 128],
                                        v_t[:, tk, 2 * jp, 0:65],
                                        start=(tk == 0), stop=(tk == ktiles - 1),
                                        skip_group_check=True)
                                    nc.tensor.matmul(
                                        o_B[:, q8, 0:65],
                                        pv[:, 1, q8 * 128:(q8 + 1) * 128],
                                        v_t[:, tk, 2 * jp + 1, 0:65],
                                        start=(tk == 0), stop=(tk == ktiles - 1),
                                        skip_group_check=True)

                            pend = []
                            for tk in range(ktiles):
                                s2 = ps_s.tile([128, 2, 512], F32, tag="sps")
                                nc.tensor.matmul(
                                    s2[:, 0, :], kf_t[0:64, jp, tk * 128:(tk + 1) * 128],
                                    qf_t[0:64, jp, tqc * 512:(tqc + 1) * 512],
                                    start=True, stop=True)
                                nc.tensor.matmul(
                                    s2[:, 1, :], kf_t[64:128, jp, tk * 128:(tk + 1) * 128],
                                    qf_t[64:128, jp, tqc * 512:(tqc + 1) * 512],
                                    start=True, stop=True)
                                if ((tk * 7) % 16) >= 7:   # ~9/16 on ACT
                                    p2 = wk.tile([128, 2, 512], F8E5, tag="p2a")
                                    nc.scalar.activation(out=p2[:], in_=s2[:],
                                                         func=AF.Exp, scale=SCALE,
                                                         bias=mln4[:])
                                    pv = p2[:]
                                else:
                                    p2i = wk.tile([128, 2, 512], I8, tag="p2i")
                                    nc.vector.tensor_scalar(
                                        out=p2i[:], in0=s2[:], scalar1=FE_A5,
                                        scalar2=FE_B5, op0=OP.mult, op1=OP.add)
                                    pv = p2i[:].bitcast(F8E5)
                                pend.append((pv, tk))
                                if len(pend) > 2:
                                    emit_o(*pend.pop(0))
                            for pv, tk in pend:
                                emit_o(pv, tk)
                            # batched normalize: one recip + one bc-mul per head
                            rec = wk.tile([128, 2, 4], F32, tag="rec")
                            nc.vector.reciprocal(rec[:, 0, :], o_A[:, :, 0])
                            nc.vector.reciprocal(rec[:, 1, :], o_B[:, :, 0])
                            otm = wk.tile([128, 4, 2, HD], BF16, tag="otm")
                            nc.vector.tensor_mul(otm[:, :, 0, :], o_A[:, :, 1:65],
                                                 _bc_inner(rec[:, 0, :], HD))
                            nc.vector.tensor_mul(otm[:, :, 1, :], o_B[:, :, 1:65],
                                                 _bc_inner(rec[:, 1, :], HD))
                            trt = ps_tr.tile([128, 512], BF16, tag="tro")
                            for q8 in range(4):
                                nc.tensor.transpose(
                                    trt[:, q8 * 128:(q8 + 1) * 128],
                                    otm[:, q8].rearrange("p a b -> p (a b)"),
                                    ident[:])
                            ti0 = tqc * 4
                            nc.any.tensor_copy(
                                of_t[:, jp, ti0 * 128:(ti0 + 4) * 128],
                                trt[:])

            def project_residual(w_dram, act_f, bias_row, out_fn):
                """out = (w^T act + bias_row) * scal + prev, via fp8 DoubleRow;
                out_fn(i, sl, ps) consumes the psum tile."""
                with tc.tile_pool(name="proj_w", bufs=1) as pw, \
                     tc.tile_pool(name="proj_ps", bufs=3, space="PSUM") as pp:
                    w_sb = pw.tile([128, CT, C], F8E4, tag="wproj")
                    nc.sync.dma_start(w_sb[:],
                                      w_dram.rearrange("(j p) o -> p j o", p=128))
                    for i in range(CT):
                        for tcx in range(2):
                            sl = slice(tcx * 512, (tcx + 1) * 512)
                            ps = pp.tile([128, 512], F32, tag="pp")
                            first = True
                            if bias_row is not None:
                                nc.tensor.matmul(ps[:],
                                                 bias_row[0:1, i * 128:(i + 1) * 128],
                                                 ones_row[:], start=True, stop=False)
                                first = False
                            for j2 in range(CT // 2):
                                nc.tensor.matmul(
                                    ps[:],
                                    w_sb[:, 2 * j2:2 * j2 + 2, i * 128:(i + 1) * 128],
                                    act_f[:, 2 * j2:2 * j2 + 2, sl],
                                    start=first and (j2 == 0),
                                    stop=(j2 == CT // 2 - 1),
                                    perf_mode=DRM)
                            out_fn(i, sl, ps)

            # ================= SA + CA-kv scope =================
            ca_hold = top.enter_context(tc.tile_pool(name="attn_ca", bufs=1))
            k_fca = ca_hold.tile([128, CT, M], F8E4, tag="kfca")
            v_ca = ca_hold.tile([128, MT, H, 68], F8E4, tag="vca")
            nc.vector.memset(v_ca[:, :, :, 0:1], 1.0)

            with tc.tile_pool(name="attn_sa", bufs=1) as attn_sa:
                q_f = attn_sa.tile([128, CT, NQ], F8E4, tag="qf")
                k_f = attn_sa.tile([128, CT, N], F8E4, tag="kf")
                v_sa = attn_sa.tile([128, NT, H, 68], F8E4, tag="vsa")
                nc.vector.memset(v_sa[:, :, :, 0:1], 1.0)
                o_f = attn_sa.tile([128, CT, NQ], F8E4, tag="of")

                # ---- phase 1: SA qkv + CA kv + LN/rope + pack ----
                with tc.tile_pool(name="p1_x", bufs=1) as p1x, \
                     tc.tile_pool(name="p1_wq", bufs=2) as p1wq, \
                     tc.tile_pool(name="p1_work", bufs=4) as work, \
                     tc.tile_pool(name="p1_ps", bufs=4, space="PSUM") as p1ps, \
                     tc.tile_pool(name="p1_tr", bufs=2, space="PSUM") as p1tr:
                    xT_sb = p1x.tile([128, CT, N], F8E4)
                    xT_r = XT.rearrange("(j p) t -> p j t", p=128)
                    for q4 in range(4):
                        sl4 = slice(q4 * 512, (q4 + 1) * 512)
                        nc.gpsimd.dma_start(xT_sb[:, :, sl4], xT_r[:, :, sl4])
                    ctx_sb = p1x.tile([128, CT, M], F8E4, tag="ctx")
                    ctx_r = CTXT.rearrange("(j p) t -> p j t", p=128)
                    for q4 in range(2):
                        sl4 = slice(q4 * 512, (q4 + 1) * 512)
                        nc.gpsimd.dma_start(ctx_sb[:, :, sl4], ctx_r[:, :, sl4])
                    wqkv_r = WQKV.rearrange("(j p) o -> p j o", p=128)
                    for ch in range(6):
                        w_ch = p1wq.tile([128, CT, 512], F8E4, tag="wch")
                        nc.sync.dma_start(w_ch[:],
                                          wqkv_r[:, :, ch * 512:(ch + 1) * 512])
                        ntile = NTQ if ch < 2 else NT
                        for i in range(ntile):
                            ps = p1ps.tile([128, 512], F32, tag="qkv")
                            for j2 in range(CT // 2):
                                nc.tensor.matmul(
                                    ps[:],
                                    xT_sb[:, 2 * j2:2 * j2 + 2, i * 128:(i + 1) * 128],
                                    w_ch[:, 2 * j2:2 * j2 + 2, :],
                                    start=(j2 == 0), stop=(j2 == CT // 2 - 1),
                                    perf_mode=DRM)
                            if ch < 2:       # q
                                ln_rope_chunk(ps, work, p1tr, ch * 8, cosq_sa,
                                              wq_sa, bq_sa, q_f, i * 128, i,
                                              alt=i)
                            elif ch < 4:     # k
                                ln_rope_chunk(ps, work, p1tr, (ch - 2) * 8,
                                              cosk_sa, wk_sa, bk_sa, k_f,
                                              i * 128, i, alt=i)
                            else:            # v
                                hs = (ch - 4) * 8
                                nc.any.tensor_copy(
                                    v_sa[:, i, hs:hs + 8, 1:65],
                                    ps[:].rearrange("p (h d) -> p h d", d=HD))
                    # CA k/v from ctx (independent of SA attention)
                    for src, is_v in ((CAWK, False), (CAWV, True)):
                        src_r = src.rearrange("(j p) o -> p j o", p=128)
                        for ch in range(2):
                            w_ch = p1wq.tile([128, CT, 512], F8E4, tag="wch")
                            nc.sync.dma_start(w_ch[:],
                                              src_r[:, :, ch * 512:(ch + 1) * 512])
                            for i in range(MT):
                                ps = p1ps.tile([128, 512], F32, tag="qkv")
                                for j2 in range(CT // 2):
                                    nc.tensor.matmul(
                                        ps[:],
                                        ctx_sb[:, 2 * j2:2 * j2 + 2, i * 128:(i + 1) * 128],
                                        w_ch[:, 2 * j2:2 * j2 + 2, :],
                                        start=(j2 == 0), stop=(j2 == CT // 2 - 1),
                                        perf_mode=DRM)
                                if not is_v:
                                    ln_rope_chunk(ps, work, p1tr, ch * 8, None,
                                                  None, None, k_fca, i * 128, i,
                                                  alt=i)
                                else:
                                    hs = ch * 8
                                    nc.any.tensor_copy(
                                        v_ca[:, i, hs:hs + 8, 1:65],
                                        ps[:].rearrange("p (h d) -> p h d", d=HD))

                # ---- phase 2: SA attention ----
                attention(k_f, v_sa, q_f, o_f, NT)

                # ---- phase 3: SA out proj + residual + CA q proj ----
                x1_f32 = residf.tile([128, CT, NQ], F32, tag="xf")
                x1_f8 = residq.tile([128, CT, NQ], F8E4, tag="xq")
                q_fca = ca_hold.tile([128, CT, NQ], F8E4, tag="qfca")
                with tc.tile_pool(name="p3_x0", bufs=3) as p3x0, \
                     tc.tile_pool(name="p4_w", bufs=2) as p4w, \
                     tc.tile_pool(name="p4_work", bufs=2) as work4, \
                     tc.tile_pool(name="p4_ps", bufs=3, space="PSUM") as p4ps, \
                     tc.tile_pool(name="p4_tr", bufs=2, space="PSUM") as p4tr:
                    def prev0(i, sl):
                        t = p3x0.tile([128, 512], F32, tag="x0")
                        nc.gpsimd.dma_start(t[:], XOWN[i * 128:(i + 1) * 128, sl])
                        return t[:]

                    def out0(i, sl, ps):
                        nc.vector.scalar_tensor_tensor(
                            out=x1_f32[:, i, sl], in0=ps[:],
                            scalar=ls0[:, i:i + 1], in1=prev0(i, sl),
                            op0=OP.mult, op1=OP.add)
                        nc.scalar.copy(x1_f8[:, i, sl], x1_f32[:, i, sl])

                    project_residual(SAWO, o_f,
                                     sabo[:] if sabo is not None else None,
                                     out0)

                    # CA q proj from x1_f8
                    cawq_r = CAWQ.rearrange("(j p) o -> p j o", p=128)
                    for ch in range(2):
                        w_ch = p4w.tile([128, CT, 512], F8E4, tag="wch4")
                        nc.sync.dma_start(w_ch[:],
                                          cawq_r[:, :, ch * 512:(ch + 1) * 512])
                        for i in range(NTQ):
                            ps = p4ps.tile([128, 512], F32, tag="kv")
                            for j2 in range(CT // 2):
                                nc.tensor.matmul(
                                    ps[:],
                                    x1_f8[:, 2 * j2:2 * j2 + 2, i * 128:(i + 1) * 128],
                                    w_ch[:, 2 * j2:2 * j2 + 2, :],
                                    start=(j2 == 0), stop=(j2 == CT // 2 - 1),
                                    perf_mode=DRM)
                            ln_rope_chunk(ps, work4, p4tr, ch * 8, cosq_ca,
                                          wq_ca, bq_ca, q_fca, i * 128, i,
                                          alt=i)

            # ================= CA attention + out proj =================
            o_fca = ca_hold.tile([128, CT, NQ], F8E4, tag="ofca")
            attention(k_fca, v_ca, q_fca, o_fca, MT)

            x2_f8 = residq.tile([128, CT, NQ], F8E4, tag="xq")

            def out1(i, sl, ps):
                # x2 = lt1*ca + x1, updating the fp32 residual in place
                nc.vector.scalar_tensor_tensor(
                    out=x1_f32[:, i, sl], in0=ps[:], scalar=lt1[:, i:i + 1],
                    in1=x1_f32[:, i, sl], op0=OP.mult, op1=OP.add)
                nc.scalar.copy(x2_f8[:, i, sl], x1_f32[:, i, sl])

            project_residual(CAWO, o_fca,
                             cabo[:] if cabo is not None else None,
                             out1)

            # ============ phase 5: SwiGLU FFN ============
            with tc.tile_pool(name="p5_hp", bufs=1) as p5hp:
                hp = p5hp.tile([128, JT, N], F8E4, tag="hp")
                w1g_r = W1G.rearrange("(j p) o -> p j o", p=128)
                w1x_r = W1X.rearrange("(j p) o -> p j o", p=128)
                w2_r = W2.rearrange("(j p) o -> p j o", p=128)
                with tc.tile_pool(name="p5_w", bufs=3) as p5w, \
                     tc.tile_pool(name="p5_work", bufs=3) as work5, \
                     tc.tile_pool(name="p5_psg", bufs=4, space="PSUM") as psg, \
                     tc.tile_pool(name="p5_psx", bufs=4, space="PSUM") as psx:
                    for j in range(JT):
                        w1g_j = p5w.tile([128, CT, 128], F8E4, tag="w1gj")
                        nc.sync.dma_start(w1g_j[:], w1g_r[:, :, j * 128:(j + 1) * 128])
                        w1x_j = p5w.tile([128, CT, 128], F8E4, tag="w1xj")
                        nc.sync.dma_start(w1x_j[:], w1x_r[:, :, j * 128:(j + 1) * 128])
                        g_ps = [psg.tile([128, 512], F32, tag="g",
                                         name=f"gps{t}") for t in range(2)]
                        x_ps = [psx.tile([128, 512], F32, tag="x",
                                         name=f"xps{t}") for t in range(2)]
                        for j2 in range(CT // 2):
                            for tcx in range(2):
                                sl = slice(tcx * 512, (tcx + 1) * 512)
                                nc.tensor.matmul(
                                    g_ps[tcx][:], w1g_j[:, 2 * j2:2 * j2 + 2, :],
                                    x2_f8[:, 2 * j2:2 * j2 + 2, sl],
                                    start=(j2 == 0), stop=(j2 == CT // 2 - 1),
                                    perf_mode=DRM)
                        for j2 in range(CT // 2):
                            for tcx in range(2):
                                sl = slice(tcx * 512, (tcx + 1) * 512)
                                nc.tensor.matmul(
                                    x_ps[tcx][:], w1x_j[:, 2 * j2:2 * j2 + 2, :],
                                    x2_f8[:, 2 * j2:2 * j2 + 2, sl],
                                    start=(j2 == 0), stop=(j2 == CT // 2 - 1),
                                    perf_mode=DRM)
                        for tcx in range(2):
                            g_sb = work5.tile([128, 512], BF16, tag="gsb")
                            nc.scalar.activation(out=g_sb[:], in_=g_ps[tcx][:],
                                                 func=AF.Silu, bias=b1g[:, j:j + 1])
                            nc.vector.scalar_tensor_tensor(
                                out=hp[:, j, tcx * 512:(tcx + 1) * 512],
                                in0=x_ps[tcx][:], scalar=b1x[:, j:j + 1],
                                in1=g_sb[:], op0=OP.add, op1=OP.mult)
                with tc.tile_pool(name="p5_w2", bufs=8) as p5w2, \
                     tc.tile_pool(name="p5_work2", bufs=3) as work52, \
                     tc.tile_pool(name="p5_psf", bufs=3, space="PSUM") as psf:
                    w2_tiles = []
                    for i in range(CT):
                        w2_i = p5w2.tile([128, JT, 128], F8E4, tag="w2i",
                                         name=f"w2i{i}")
                        nc.sync.dma_start(w2_i[:], w2_r[:, :, i * 128:(i + 1) * 128])
                        w2_tiles.append(w2_i)
                    for i in range(CT):
                        w2_i = w2_tiles[i]
                        for tcx in range(2):
                            sl = slice(tcx * 512, (tcx + 1) * 512)
                            f_ps = psf.tile([128, 512], F32, tag="f")
                            first = True
                            if b2r is not None:
                                nc.tensor.matmul(f_ps[:],
                                                 b2r[0:1, i * 128:(i + 1) * 128],
                                                 ones_row[:], start=True, stop=False)
                                first = False
                            for j2 in range(JT // 2):
                                nc.tensor.matmul(
                                    f_ps[:], w2_i[:, 2 * j2:2 * j2 + 2, :],
                                    hp[:, 2 * j2:2 * j2 + 2, sl],
                                    start=first and (j2 == 0),
                                    stop=(j2 == JT // 2 - 1),
                                    perf_mode=DRM)
                            y_sb = work52.tile([128, 512], F32, tag="ysb")
                            nc.vector.scalar_tensor_tensor(
                                out=y_sb[:], in0=f_ps[:], scalar=ls2[:, i:i + 1],
                                in1=x1_f32[:, i, sl], op0=OP.mult, op1=OP.add)
                            nc.gpsimd.dma_start(Y[i * 128:(i + 1) * 128, sl], y_sb[:])

    nc.compile()
    return nc


def _rope_tables(rope, g, b):
    """cos/W (swap-multiplier) tables with gamma and the rstd 1/8-fold
    (tables x8); plus additive beta table (or None, unscaled)."""
    sin, cos = rope[:, :HD], rope[:, HD:]
    W = np.empty_like(sin)
    W[:, 0::2] = sin[:, 1::2]
    W[:, 1::2] = -sin[:, 0::2]
    c1 = (cos * g[None, :] * 8.0).astype(np.float32)
    w1 = (W * g[None, :] * 8.0).astype(np.float32)
    bt = None
    if b is not None and np.any(b):
        bw = b[None, :] * W
        bwsw = np.empty_like(bw)
        bwsw[:, 0::2], bwsw[:, 1::2] = bw[:, 1::2], bw[:, 0::2]
        bt = (b[None, :] * cos + bwsw).astype(np.float32)
    return np.ascontiguousarray(c1), np.ascontiguousarray(w1), bt


def _center_heads(w):
    """Subtract per-64-wide-head-block column mean: x@w then has exactly
    zero mean per head, so LN's mean subtraction is a no-op (LN is
    shift-invariant, so this does not change the reference math)."""
    w = np.asarray(w, np.float32).copy()
    for h0 in range(0, w.shape[1], HD):
        blk = w[:, h0:h0 + HD]
        blk -= blk.mean(axis=1, keepdims=True)
    return w


def _f8(a):
    return np.clip(np.asarray(a, np.float32), -240, 240).astype(
        ml_dtypes.float8_e4m3)


def _prepare(inputs):
    """Host-side sharding: returns (flags, in_maps) for the 8 cores."""
    f32 = np.float32
    bf = ml_dtypes.bfloat16
    x = np.asarray(inputs["x"], f32)
    ctx = np.asarray(inputs["ctx"], f32)
    rope = np.asarray(inputs["rope"], f32)

    flags = {
        "bq_sa": bool(np.any(inputs["sa_qb"])),
        "bk_sa": bool(np.any(inputs["sa_kb"])),
        "bq_ca": bool(np.any(inputs["ca_qb"])),
        "cakb": bool(np.any(inputs["ca_kb"])),
        "sa_bo": bool(np.any(inputs["sa_bo"])),
        "ca_bo": bool(np.any(inputs["ca_bo"])),
        "b2": bool(np.any(inputs["b2"])),
    }

    # guard the fp8e5m2 softmax fast path: |s*SCALE| must stay under ~9.1
    gq = np.asarray(inputs["sa_qg"], f32)
    gk = np.asarray(inputs["sa_kg"], f32)
    bq = np.asarray(inputs["sa_qb"], f32)
    bk = np.asarray(inputs["sa_kb"], f32)
    bound = 1.07 * (8 * np.abs(gq).max() + 8 * np.abs(bq).max()) * \
        (8 * np.abs(gk).max() + 8 * np.abs(bk).max()) / 8.0
    assert bound < 10.4, f"qk-norm bound {bound} too large for fp8 softmax"

    def fm(v, nt):  # feature-major [128, nt]
        return np.ascontiguousarray(np.asarray(v, f32).reshape(nt, 128).T)

    wqkv = np.asarray(inputs["wqkv"], f32).copy()
    wqkv[:, :C] = _center_heads(wqkv[:, :C])
    wqkv[:, C:2 * C] = _center_heads(wqkv[:, C:2 * C])

    shared = {
        "wqkv": _f8(wqkv),
        "sa_wo": _f8(inputs["sa_wo"]),
        "ca_wq": _f8(_center_heads(inputs["ca_wq"])),
        "ca_wk": _f8(_center_heads(inputs["ca_wk"])),
        "ca_wv": _f8(inputs["ca_wv"]),
        "ca_wo": _f8(inputs["ca_wo"]),
        "w1g": _f8(inputs["w1g"]),
        "w1x": _f8(inputs["w1x"]),
        "w2": _f8(inputs["w2"]),
        "b1g_f": fm(inputs["b1g"], JT),
        "b1x_f": fm(inputs["b1x"], JT),
        "ls0_f": fm(inputs["ls0"], CT),
        "ls1_f": fm(inputs["ls1"], CT),
        "ls2_f": fm(inputs["ls2"], CT),
        "gate_f": fm(inputs["ca_gate"], CT),
        "cakg": np.ascontiguousarray(np.tile(
            np.asarray(inputs["ca_kg"], f32)[None, :] * 8.0,
            (128, 1))).astype(bf),
    }
    if flags["sa_bo"]:
        shared["sa_bo_row"] = np.asarray(inputs["sa_bo"], f32).reshape(1, C).astype(bf)
    if flags["ca_bo"]:
        shared["ca_bo_row"] = np.asarray(inputs["ca_bo"], f32).reshape(1, C).astype(bf)
    if flags["b2"]:
        shared["b2_row"] = np.asarray(inputs["b2"], f32).reshape(1, C).astype(bf)
    if flags["cakb"]:
        shared["cakb"] = np.ascontiguousarray(
            np.tile(np.asarray(inputs["ca_kb"], f32)[None, :], (128, 1)))

    cq_sa, wq_sa, bq_sa = _rope_tables(rope, np.asarray(inputs["sa_qg"], f32),
                                       np.asarray(inputs["sa_qb"], f32))
    ck_sa, wk_sa, bk_sa = _rope_tables(rope, np.asarray(inputs["sa_kg"], f32),
                                       np.asarray(inputs["sa_kb"], f32))
    cq_ca, wq_ca, bq_ca = _rope_tables(rope, np.asarray(inputs["ca_qg"], f32),
                                       np.asarray(inputs["ca_qb"], f32))

    in_maps = []
    for core in range(8):
        b, h = divmod(core, 2)
        own = slice(h * NQ, (h + 1) * NQ)
        oth = slice((1 - h) * NQ, (2 - h) * NQ)
        perm = np.r_[own, oth]
        xp = x[b][perm]                      # [2048, 1024] own rows first
        m = dict(shared)
        m["xT"] = _f8(np.ascontiguousarray(xp.T))
        m["x_own"] = np.ascontiguousarray(x[b][own].T)
        m["ctxT"] = _f8(np.ascontiguousarray(ctx[b].T))
        m["cosq_sa"] = cq_sa[own].astype(bf)
        m["wq_sa"] = wq_sa[own].astype(bf)
        m["cosk_sa"] = np.ascontiguousarray(ck_sa[perm]).astype(bf)
        m["wk_sa"] = np.ascontiguousarray(wk_sa[perm]).astype(bf)
        m["cosq_ca"] = cq_ca[own].astype(bf)
        m["wq_ca"] = wq_ca[own].astype(bf)
        if flags["bq_sa"]:
            m["bq_sa"] = bq_sa[own]
        if flags["bk_sa"]:
            m["bk_sa"] = np.ascontiguousarray(bk_sa[perm])
        if flags["bq_ca"]:
            m["bq_ca"] = bq_ca[own]
        in_maps.append(m)
    return flags, in_maps


def _get_program(flags):
    key = tuple(sorted(flags.items()))
    if key not in _CACHE:
        _CACHE[key] = _build_program(flags)
    return _CACHE[key]


def _run(in_maps, nc, trace=False, trace_kwargs=None):
    return run_bass_kernel_spmd(nc, in_maps, list(range(8)), trace=trace,
                                **(trace_kwargs or {}))


def kernel(**inputs):
    flags, in_maps = _prepare(inputs)
    nc = _get_program(flags)
    res = _run(in_maps, nc)
    out = np.empty((B, N, C), np.float32)
    for core in range(8):
        b, h = divmod(core, 2)
        out[b, h * NQ:(h + 1) * NQ, :] = res.results[core]["y"].T
    return out


# revision 30
# speedup vs baseline: 1.2081x; 1.2081x over previous
"""Trainium2 Bass kernel for a cross-attention transformer block.

Sharding: 8 cores = 4 batches x 2 query-row halves (pure data parallel,
no collectives). Each core computes the full block for its 1024 query
tokens, duplicating only the K/V projections for the other half's rows.

v2 (fp8): all large GEMMs run fp8e4m3 with DoubleRow perf mode
(2 k-tiles contracted per matmul); attention probabilities are fp8e5m2
(scaled by 1/4, exact-cancelling in the softmax ratio) so the o-matmul
weight loads run at FWL fp8 rate; q/k LN mean-subtraction is folded into
host-side centered projection weights (LN(q - c) == LN(q)); the LN+rope
chain runs bf16 after a single fp32 PSUM read; softmax denominators are
normalized with two batched broadcast multiplies per head-pair chunk.

On-device layout convention:
  - residual stream kept feature-major [C(part), tokens(free)] in fp32
  - matmuls fp8 (fp32 PSUM accumulation)
  - attention q/k LN + RoPE done token-major [tokens(part), 64(free)],
    then PE-transposed to feature-major per head
  - softmax without max subtraction (qk-layernorm bounds |s*scale| < 9.1
    incl. fp8 margin; p = exp(s*scale)/4 fits fp8e5m2 with headroom)
"""

import os
import sys
import contextlib

for _p in ("/opt/trn_rl_repo",):
    if os.path.isdir(_p) and _p not in sys.path:
        sys.path.append(_p)

import numpy as np
import ml_dtypes

import concourse.bass as bass
import concourse.mybir as mybir
import concourse.tile as tile
from concourse import bacc
from concourse.bass_utils import run_bass_kernel_spmd
from concourse.masks import make_identity

BF16 = mybir.dt.bfloat16
F32 = mybir.dt.float32
F8E4 = mybir.dt.float8e4
F8E5 = mybir.dt.float8e5
I8 = mybir.dt.int8
AF = mybir.ActivationFunctionType
OP = mybir.AluOpType
DRM = mybir.MatmulPerfMode.DoubleRow

B, N, M, C, H = 4, 2048, 1024, 1024, 16
HD = C // H            # 64
HID = 4 * C            # 4096
SCALE = 1.0 / np.sqrt(HD)
EPS = 1e-6
NQ = N // 2            # own query tokens per core (1024)
NT = N // 128          # 16 token tiles of full seq
NTQ = NQ // 128        # 8 own token tiles
MT = M // 128          # 8 ctx token tiles
CT = C // 128          # 8 feature tiles
JT = HID // 128        # 32 hidden tiles

# exp -> fp8e5m2, probabilities scaled by 2^-EXP_SHIFT (cancels in ratio)
EXP_SHIFT = 2
FE_A5 = float(4.0 * SCALE / np.log(2.0))
FE_B5 = float(4.0 * (15 - EXP_SHIFT) - 0.172 + 0.5)

_CACHE = {}


def _build_program(flags):
    """Build the single-core Tile program. `flags` controls optional beta
    and bias paths (all-zero ones are skipped)."""
    nc = bacc.Bacc("TRN2", target_bir_lowering=False, debug=False)

    def din(name, shape, dt):
        return nc.dram_tensor(name, list(shape), dt, kind="ExternalInput").ap()

    # --- DRAM inputs (per core) ---
    XT = din("xT", (C, N), F8E4)                 # x[b].T fp8, own rows first
    XOWN = din("x_own", (C, NQ), F32)            # fp32 residual basis
    CTXT = din("ctxT", (C, M), F8E4)
    WQKV = din("wqkv", (C, 3 * C), F8E4)         # q/k cols centered per head
    SAWO = din("sa_wo", (C, C), F8E4)
    CAWQ = din("ca_wq", (C, C), F8E4)
    CAWK = din("ca_wk", (C, C), F8E4)
    CAWV = din("ca_wv", (C, C), F8E4)
    CAWO = din("ca_wo", (C, C), F8E4)
    W1G = din("w1g", (C, HID), F8E4)
    W1X = din("w1x", (C, HID), F8E4)
    W2 = din("w2", (HID, C), F8E4)
    COSQ_SA = din("cosq_sa", (NQ, HD), BF16)     # cos*g*8 tables
    WQ_SA = din("wq_sa", (NQ, HD), BF16)
    COSK_SA = din("cosk_sa", (N, HD), BF16)
    WK_SA = din("wk_sa", (N, HD), BF16)
    COSQ_CA = din("cosq_ca", (NQ, HD), BF16)
    WQ_CA = din("wq_ca", (NQ, HD), BF16)
    CAKG = din("cakg", (128, HD), BF16)          # gamma*8 row (bc over heads)
    B1GF = din("b1g_f", (128, JT), F32)
    B1XF = din("b1x_f", (128, JT), F32)
    LS0 = din("ls0_f", (128, CT), F32)
    LS1 = din("ls1_f", (128, CT), F32)
    LS2 = din("ls2_f", (128, CT), F32)
    GATE = din("gate_f", (128, CT), F32)
    SABO = din("sa_bo_row", (1, C), BF16) if flags["sa_bo"] else None
    CABO = din("ca_bo_row", (1, C), BF16) if flags["ca_bo"] else None
    B2R = din("b2_row", (1, C), BF16) if flags["b2"] else None
    BQ_SA = din("bq_sa", (NQ, HD), F32) if flags["bq_sa"] else None
    BK_SA = din("bk_sa", (N, HD), F32) if flags["bk_sa"] else None
    BQ_CA = din("bq_ca", (NQ, HD), F32) if flags["bq_ca"] else None
    CAKB = din("cakb", (128, HD), F32) if flags["cakb"] else None

    Y = nc.dram_tensor("y", [C, NQ], F32, kind="ExternalOutput").ap()

    with tile.TileContext(nc) as tc:
        with contextlib.ExitStack() as top:
            consts = top.enter_context(tc.tile_pool(name="consts", bufs=1))
            residf = top.enter_context(tc.tile_pool(name="residf", bufs=1))
            residq = top.enter_context(tc.tile_pool(name="residq", bufs=1))

            # ---- constants ----
            ident = consts.tile([128, 128], BF16)
            make_identity(nc, ident[:])
            eps_t = consts.tile([128, 1], F32)
            nc.vector.memset(eps_t[:], EPS * HD)     # folded: sqrt(ssq+64eps)
            mln4 = consts.tile([128, 1], F32)
            nc.vector.memset(mln4[:], float(-np.log(2.0 ** EXP_SHIFT)))
            ones_row = consts.tile([1, 512], BF16)
            nc.vector.memset(ones_row[:], 1.0)

            def load_const(ap_in, shape, dt, tag):
                t = consts.tile(list(shape), dt, tag=tag)
                nc.sync.dma_start(t[:], ap_in)
                return t

            ls0 = load_const(LS0[:], (128, CT), F32, "ls0")
            ls1 = load_const(LS1[:], (128, CT), F32, "ls1")
            ls2 = load_const(LS2[:], (128, CT), F32, "ls2")
            gate = load_const(GATE[:], (128, CT), F32, "gate")
            b1g = load_const(B1GF[:], (128, JT), F32, "b1g")
            b1x = load_const(B1XF[:], (128, JT), F32, "b1x")
            cakg = load_const(CAKG[:], (128, HD), BF16, "cakg")
            sabo = load_const(SABO[:], (1, C), BF16, "sabo") if SABO is not None else None
            cabo = load_const(CABO[:], (1, C), BF16, "cabo") if CABO is not None else None
            b2r = load_const(B2R[:], (1, C), BF16, "b2r") if B2R is not None else None
            cakb = (load_const(CAKB[:], (128, HD), F32, "cakb")
                    if CAKB is not None else None)

            # rope tables: [128, ntiles, 64]
            def load_tab(ap_in, ntile, tag, dt=BF16):
                t = consts.tile([128, ntile, HD], dt, tag=tag)
                nc.sync.dma_start(t[:], ap_in.rearrange("(i p) d -> p i d", p=128))
                return t

            cosq_sa = load_tab(COSQ_SA[:], NTQ, "cosqsa")
            wq_sa = load_tab(WQ_SA[:], NTQ, "wqsa")
            cosk_sa = load_tab(COSK_SA[:], NT, "cosksa")
            wk_sa = load_tab(WK_SA[:], NT, "wksa")
            cosq_ca = load_tab(COSQ_CA[:], NTQ, "cosqca")
            wq_ca = load_tab(WQ_CA[:], NTQ, "wqca")
            bq_sa = load_tab(BQ_SA[:], NTQ, "bqsa", F32) if BQ_SA is not None else None
            bk_sa = load_tab(BK_SA[:], NT, "bksa", F32) if BK_SA is not None else None
            bq_ca = load_tab(BQ_CA[:], NTQ, "bqca", F32) if BQ_CA is not None else None

            # lt1 = ls1 * tanh(gate)
            th = consts.tile([128, CT], F32)
            nc.scalar.activation(out=th[:], in_=gate[:], func=AF.Tanh)
            lt1 = consts.tile([128, CT], F32)
            nc.vector.tensor_mul(lt1[:], ls1[:], th[:])

            # ============ helpers ============
            def _bc_heads(ap2):
                """[128, 64] table -> [128, 8, 64] broadcast view (step-0)."""
                return bass.AP(tensor=ap2.tensor, offset=ap2.offset,
                               ap=[list(ap2.ap[0]), [0, 8], list(ap2.ap[1])])

            def _bc_inner(ap2, n):
                """[128, k] per-head scalars -> [128, k, n] broadcast view."""
                return bass.AP(tensor=ap2.tensor, offset=ap2.offset,
                               ap=[list(ap2.ap[0]), list(ap2.ap[1]), [0, n]])

            def _swap512(ap2):
                """[128, 512] -> pair-swapped view [128, 256, 2]."""
                return bass.AP(tensor=ap2.tensor, offset=ap2.offset + 1,
                               ap=[list(ap2.ap[0]), [2, 256], [-1, 2]])

            def ln_rope_chunk(ps, work, trps, heads0, cos_t, w_t, b_t,
                              dest, dest_col, tabi, alt=0):
                """LN + RoPE on a [128, 512] psum chunk (8 heads; mean is
                pre-folded into centered weights), writing paired-transposed
                fp8 into dest[:, jp, dest_col:+128].
                cos_t None => no rope (plain gamma via cakg)."""
                ssq = work.tile([128, 8], F32, tag="ssq")
                sq = work.tile([128, 512], BF16, tag="wsq")
                nc.scalar.square(sq[:], ps[:])
                nc.vector.reduce_sum(out=ssq[:], in_=sq[:].rearrange(
                    "p (h d) -> p h d", d=HD), axis=mybir.AxisListType.X)
                std8 = work.tile([128, 8], F32, tag="std8")
                nc.scalar.activation(out=std8[:], in_=ssq[:], func=AF.Sqrt,
                                     bias=eps_t[:])
                r8 = work.tile([128, 8], F32, tag="r8")
                nc.vector.reciprocal(r8[:], std8[:])
                # t0 = ps * (1/std8)  (true qn/8; the 8 is folded in tables)
                t0 = work.tile([128, 512], BF16, tag="wt0")
                nc.vector.tensor_mul(t0[:].rearrange("p (h d) -> p h d", d=HD),
                                     ps[:].rearrange("p (h d) -> p h d", d=HD),
                                     _bc_inner(r8[:], HD))
                qr = work.tile([128, 512], BF16, tag="qr")
                if cos_t is not None:
                    t1 = work.tile([128, 512], BF16, tag="wt1")
                    nc.vector.tensor_mul(t1[:].rearrange("p (h d) -> p h d", d=HD),
                                         t0[:].rearrange("p (h d) -> p h d", d=HD),
                                         _bc_heads(cos_t[:, tabi, :]))
                    t2 = work.tile([128, 512], BF16, tag="wt2")
                    nc.vector.tensor_mul(t2[:].rearrange("p (h d) -> p h d", d=HD),
                                         t0[:].rearrange("p (h d) -> p h d", d=HD),
                                         _bc_heads(w_t[:, tabi, :]))
                    if b_t is None:
                        nc.vector.tensor_add(qr[:].rearrange("p (a b) -> p a b", b=2),
                                             t1[:].rearrange("p (a b) -> p a b", b=2),
                                             _swap512(t2[:]))
                    else:
                        t3 = work.tile([128, 512], BF16, tag="wt3")
                        nc.vector.tensor_add(t3[:].rearrange("p (a b) -> p a b", b=2),
                                             t1[:].rearrange("p (a b) -> p a b", b=2),
                                             _swap512(t2[:]))
                        nc.vector.tensor_add(qr[:].rearrange(
                            "p (h d) -> p h d", d=HD), t3[:].rearrange(
                            "p (h d) -> p h d", d=HD), _bc_heads(b_t[:, tabi, :]))
                else:
                    # CA k: gamma (+ beta) broadcast over heads
                    if cakb is None:
                        nc.vector.tensor_mul(qr[:].rearrange("p (h d) -> p h d", d=HD),
                                             t0[:].rearrange("p (h d) -> p h d", d=HD),
                                             _bc_heads(cakg[:]))
                    else:
                        t3 = work.tile([128, 512], BF16, tag="wt3")
                        nc.vector.tensor_mul(t3[:].rearrange("p (h d) -> p h d", d=HD),
                                             t0[:].rearrange("p (h d) -> p h d", d=HD),
                                             _bc_heads(cakg[:]))
                        nc.vector.tensor_add(qr[:].rearrange("p (h d) -> p h d", d=HD),
                                             t3[:].rearrange("p (h d) -> p h d", d=HD),
                                             _bc_heads(cakb[:]))
                # paired transposes: [128 t, 128 (dA|dB)] -> [128 d-pair, 128 t]
                trt = trps.tile([128, 512], BF16, tag="trq")
                for jp2 in range(4):
                    nc.tensor.transpose(trt[:, jp2 * 128:(jp2 + 1) * 128],
                                        qr[:, jp2 * 128:(jp2 + 1) * 128],
                                        ident[:])
                jp0 = heads0 // 2
                nc.any.tensor_copy(
                    dest[:, jp0:jp0 + 4, dest_col:dest_col + 128],
                    trt[:].rearrange("p (j t) -> p j t", t=128))

            def attention(kf_t, v_t, qf_t, of_t, ktiles):
                """Pair-fused attention: for each head pair, s^T = k^T q into a
                2-bank psum, p = exp(scale s)/4 in fp8e5m2 alternating ACT /
                DVE-int8-fast-exp2, o = p^T v_aug with ones-column
                denominators, batched normalize, pair-transpose to of_t."""
                with tc.tile_pool(name="att_ps", bufs=2, space="PSUM") as ps_s, \
                     tc.tile_pool(name="att_po", bufs=3, space="PSUM") as ps_o, \
                     tc.tile_pool(name="att_tr", bufs=1, space="PSUM") as ps_tr, \
                     tc.tile_pool(name="att_wk", bufs=6) as wk:
                    for jp in range(CT):
                        for tqc in range(2):
                            o_A = ps_o.tile([128, 4, 128], F32, tag="ops")
                            o_B = ps_o.tile([128, 4, 128], F32, tag="ops")

                            def emit_o(pv, tk):
                                for q8 in range(4):
                                    nc.tensor.matmul(
                                        o_A[:, q8, 0:65],
                                        pv[:, 0, q8 * 128:(q8 + 1) * 128],
                                        v_t[:, tk, 2 * jp, 0:65],
                                        start=(tk == 0), stop=(tk == ktiles - 1),
                                        skip_group_check=True)
                                    nc.tensor.matmul(
                                        o_B[:, q8, 0:65],
                                        pv[:, 1, q8 * 128:(q8 + 1) * 128],
                                        v_t[:, tk, 2 * jp + 1, 0:65],
                                        start=(tk == 0), stop=(tk == ktiles - 1),
                                        skip_group_check=True)

                            pend = []
                            for tk in range(ktiles):
                                s2 = ps_s.tile([128, 2, 512], F32, tag="sps")
                                nc.tensor.matmul(
                                    s2[:, 0, :], kf_t[0:64, jp, tk * 128:(tk + 1) * 128],
                                    qf_t[0:64, jp, tqc * 512:(tqc + 1) * 512],
                                    start=True, stop=True)
                                nc.tensor.matmul(
                                    s2[:, 1, :], kf_t[64:128, jp, tk * 128:(tk + 1) * 128],
                                    qf_t[64:128, jp, tqc * 512:(tqc + 1) * 512],
                                    start=True, stop=True)
                                if ((tk * 7) % 16) >= 7:   # ~9/16 on ACT
                                    p2 = wk.tile([128, 2, 512], F8E5, tag="p2a")
                                    nc.scalar.activation(out=p2[:], in_=s2[:],
                                                         func=AF.Exp, scale=SCALE,
                                                         bias=mln4[:])
                                    pv = p2[:]
                                else:
                                    p2i = wk.tile([128, 2, 512], I8, tag="p2i")
                                    nc.vector.tensor_scalar(
                                        out=p2i[:], in0=s2[:], scalar1=FE_A5,
                                        scalar2=FE_B5, op0=OP.mult, op1=OP.add)
                                    pv = p2i[:].bitcast(F8E5)
                                pend.append((pv, tk))
                                if len(pend) > 2:
                                    emit_o(*pend.pop(0))
                            for pv, tk in pend:
                                emit_o(pv, tk)
                            # batched normalize: one recip + one bc-mul per head
                            rec = wk.tile([128, 2, 4], F32, tag="rec")
                            nc.vector.reciprocal(rec[:, 0, :], o_A[:, :, 0])
                            nc.vector.reciprocal(rec[:, 1, :], o_B[:, :, 0])
                            otm = wk.tile([128, 4, 2, HD], BF16, tag="otm")
                            nc.vector.tensor_mul(otm[:, :, 0, :], o_A[:, :, 1:65],
                                                 _bc_inner(rec[:, 0, :], HD))
                            nc.vector.tensor_mul(otm[:, :, 1, :], o_B[:, :, 1:65],
                                                 _bc_inner(rec[:, 1, :], HD))
                            trt = ps_tr.tile([128, 512], BF16, tag="tro")
                            for q8 in range(4):
                                nc.tensor.transpose(
                                    trt[:, q8 * 128:(q8 + 1) * 128],
                                    otm[:, q8].rearrange("p a b -> p (a b)"),
                                    ident[:])
                            ti0 = tqc * 4
                            nc.any.tensor_copy(
                                of_t[:, jp, ti0 * 128:(ti0 + 4) * 128],
                                trt[:])

            def project_residual(w_dram, act_f, bias_row, out_fn):
                """out = (w^T act + bias_row) * scal + prev, via fp8 DoubleRow;
                out_fn(i, sl, ps) consumes the psum tile."""
                with tc.tile_pool(name="proj_w", bufs=1) as pw, \
                     tc.tile_pool(name="proj_ps", bufs=3, space="PSUM") as pp:
                    w_sb = pw.tile([128, CT, C], F8E4, tag="wproj")
                    nc.sync.dma_start(w_sb[:],
                                      w_dram.rearrange("(j p) o -> p j o", p=128))
                    for tcx in range(2):
                        for i in range(CT):
                            sl = slice(tcx * 512, (tcx + 1) * 512)
                            ps = pp.tile([128, 512], F32, tag="pp")
                            first = True
                            if bias_row is not None:
                                nc.tensor.matmul(ps[:],
                                                 bias_row[0:1, i * 128:(i + 1) * 128],
                                                 ones_row[:], start=True, stop=False)
                                first = False
                            for j2 in range(CT // 2):
                                nc.tensor.matmul(
                                    ps[:],
                                    w_sb[:, 2 * j2:2 * j2 + 2, i * 128:(i + 1) * 128],
                                    act_f[:, 2 * j2:2 * j2 + 2, sl],
                                    start=first and (j2 == 0),
                                    stop=(j2 == CT // 2 - 1),
                                    perf_mode=DRM)
                            out_fn(i, sl, ps)

            # ================= SA + CA-kv scope =================
            ca_hold = top.enter_context(tc.tile_pool(name="attn_ca", bufs=1))
            k_fca = ca_hold.tile([128, CT, M], F8E4, tag="kfca")
            v_ca = ca_hold.tile([128, MT, H, 68], F8E4, tag="vca")
            nc.vector.memset(v_ca[:, :, :, 0:1], 1.0)

            with tc.tile_pool(name="attn_sa", bufs=1) as attn_sa:
                q_f = attn_sa.tile([128, CT, NQ], F8E4, tag="qf")
                k_f = attn_sa.tile([128, CT, N], F8E4, tag="kf")
                v_sa = attn_sa.tile([128, NT, H, 68], F8E4, tag="vsa")
                nc.vector.memset(v_sa[:, :, :, 0:1], 1.0)
                o_f = attn_sa.tile([128, CT, NQ], F8E4, tag="of")

                # ---- phase 1: SA qkv + CA kv + LN/rope + pack ----
                with tc.tile_pool(name="p1_x", bufs=1) as p1x, \
                     tc.tile_pool(name="p1_wq", bufs=2) as p1wq, \
                     tc.tile_pool(name="p1_work", bufs=4) as work, \
                     tc.tile_pool(name="p1_ps", bufs=4, space="PSUM") as p1ps, \
                     tc.tile_pool(name="p1_tr", bufs=2, space="PSUM") as p1tr:
                    xT_sb = p1x.tile([128, CT, N], F8E4)
                    xT_r = XT.rearrange("(j p) t -> p j t", p=128)
                    for q4 in range(4):
                        sl4 = slice(q4 * 512, (q4 + 1) * 512)
                        nc.gpsimd.dma_start(xT_sb[:, :, sl4], xT_r[:, :, sl4])
                    ctx_sb = p1x.tile([128, CT, M], F8E4, tag="ctx")
                    ctx_r = CTXT.rearrange("(j p) t -> p j t", p=128)
                    for q4 in range(2):
                        sl4 = slice(q4 * 512, (q4 + 1) * 512)
                        nc.gpsimd.dma_start(ctx_sb[:, :, sl4], ctx_r[:, :, sl4])
                    wqkv_r = WQKV.rearrange("(j p) o -> p j o", p=128)
                    for ch in range(6):
                        w_ch = p1wq.tile([128, CT, 512], F8E4, tag="wch")
                        nc.sync.dma_start(w_ch[:],
                                          wqkv_r[:, :, ch * 512:(ch + 1) * 512])
                        ntile = NTQ if ch < 2 else NT
                        for i in range(ntile):
                            ps = p1ps.tile([128, 512], F32, tag="qkv")
                            for j2 in range(CT // 2):
                                nc.tensor.matmul(
                                    ps[:],
                                    xT_sb[:, 2 * j2:2 * j2 + 2, i * 128:(i + 1) * 128],
                                    w_ch[:, 2 * j2:2 * j2 + 2, :],
                                    start=(j2 == 0), stop=(j2 == CT // 2 - 1),
                                    perf_mode=DRM)
                            if ch < 2:       # q
                                ln_rope_chunk(ps, work, p1tr, ch * 8, cosq_sa,
                                              wq_sa, bq_sa, q_f, i * 128, i,
                                              alt=i)
                            elif ch < 4:     # k
                                ln_rope_chunk(ps, work, p1tr, (ch - 2) * 8,
                                              cosk_sa, wk_sa, bk_sa, k_f,
                                              i * 128, i, alt=i)
                            else:            # v
                                hs = (ch - 4) * 8
                                nc.any.tensor_copy(
                                    v_sa[:, i, hs:hs + 8, 1:65],
                                    ps[:].rearrange("p (h d) -> p h d", d=HD))
                    # CA k/v from ctx (independent of SA attention)
                    for src, is_v in ((CAWK, False), (CAWV, True)):
                        src_r = src.rearrange("(j p) o -> p j o", p=128)
                        for ch in range(2):
                            w_ch = p1wq.tile([128, CT, 512], F8E4, tag="wch")
                            nc.sync.dma_start(w_ch[:],
                                              src_r[:, :, ch * 512:(ch + 1) * 512])
                            for i in range(MT):
                                ps = p1ps.tile([128, 512], F32, tag="qkv")
                                for j2 in range(CT // 2):
                                    nc.tensor.matmul(
                                        ps[:],
                                        ctx_sb[:, 2 * j2:2 * j2 + 2, i * 128:(i + 1) * 128],
                                        w_ch[:, 2 * j2:2 * j2 + 2, :],
                                        start=(j2 == 0), stop=(j2 == CT // 2 - 1),
                                        perf_mode=DRM)
                                if not is_v:
                                    ln_rope_chunk(ps, work, p1tr, ch * 8, None,
                                                  None, None, k_fca, i * 128, i,
                                                  alt=i)
                                else:
                                    hs = ch * 8
                                    nc.any.tensor_copy(
                                        v_ca[:, i, hs:hs + 8, 1:65],
                                        ps[:].rearrange("p (h d) -> p h d", d=HD))

                # ---- phase 2: SA attention ----
                attention(k_f, v_sa, q_f, o_f, NT)

                # ---- phase 3: SA out proj + residual + CA q proj ----
                x1_f32 = residf.tile([128, CT, NQ], F32, tag="xf")
                x1_f8 = residq.tile([128, CT, NQ], F8E4, tag="xq")
                q_fca = ca_hold.tile([128, CT, NQ], F8E4, tag="qfca")
                with tc.tile_pool(name="p3_x0", bufs=3) as p3x0, \
                     tc.tile_pool(name="p4_w", bufs=2) as p4w, \
                     tc.tile_pool(name="p4_work", bufs=2) as work4, \
                     tc.tile_pool(name="p4_ps", bufs=3, space="PSUM") as p4ps, \
                     tc.tile_pool(name="p4_tr", bufs=2, space="PSUM") as p4tr:
                    def prev0(i, sl):
                        t = p3x0.tile([128, 512], F32, tag="x0")
                        nc.gpsimd.dma_start(t[:], XOWN[i * 128:(i + 1) * 128, sl])
                        return t[:]

                    def out0(i, sl, ps):
                        nc.vector.scalar_tensor_tensor(
                            out=x1_f32[:, i, sl], in0=ps[:],
                            scalar=ls0[:, i:i + 1], in1=prev0(i, sl),
                            op0=OP.mult, op1=OP.add)
                        nc.scalar.copy(x1_f8[:, i, sl], x1_f32[:, i, sl])

                    project_residual(SAWO, o_f,
                                     sabo[:] if sabo is not None else None,
                                     out0)

                    # CA q proj from x1_f8
                    cawq_r = CAWQ.rearrange("(j p) o -> p j o", p=128)
                    for ch in range(2):
                        w_ch = p4w.tile([128, CT, 512], F8E4, tag="wch4")
                        nc.sync.dma_start(w_ch[:],
                                          cawq_r[:, :, ch * 512:(ch + 1) * 512])
                        for i in range(NTQ):
                            ps = p4ps.tile([128, 512], F32, tag="kv")
                            for j2 in range(CT // 2):
                                nc.tensor.matmul(
                                    ps[:],
                                    x1_f8[:, 2 * j2:2 * j2 + 2, i * 128:(i + 1) * 128],
                                    w_ch[:, 2 * j2:2 * j2 + 2, :],
                                    start=(j2 == 0), stop=(j2 == CT // 2 - 1),
                                    perf_mode=DRM)
                            ln_rope_chunk(ps, work4, p4tr, ch * 8, cosq_ca,
                                          wq_ca, bq_ca, q_fca, i * 128, i,
                                          alt=i)

            # ================= CA attention + out proj =================
            o_fca = ca_hold.tile([128, CT, NQ], F8E4, tag="ofca")
            attention(k_fca, v_ca, q_fca, o_fca, MT)

            x2_f8 = residq.tile([128, CT, NQ], F8E4, tag="xq")

            def out1(i, sl, ps):
                # x2 = lt1*ca + x1, updating the fp32 residual in place
                nc.vector.scalar_tensor_tensor(
                    out=x1_f32[:, i, sl], in0=ps[:], scalar=lt1[:, i:i + 1],
                    in1=x1_f32[:, i, sl], op0=OP.mult, op1=OP.add)
                nc.scalar.copy(x2_f8[:, i, sl], x1_f32[:, i, sl])

            project_residual(CAWO, o_fca,
                             cabo[:] if cabo is not None else None,
                             out1)

            # ============ phase 5: SwiGLU FFN ============
            with tc.tile_pool(name="p5_hp", bufs=1) as p5hp, \
                 tc.tile_pool(name="p5_w2", bufs=8) as p5w2:
                hp = p5hp.tile([128, JT, N], F8E4, tag="hp")
                w1g_r = W1G.rearrange("(j p) o -> p j o", p=128)
                w1x_r = W1X.rearrange("(j p) o -> p j o", p=128)
                w2_r = W2.rearrange("(j p) o -> p j o", p=128)
                w2_tiles = []
                for i in range(CT):
                    w2_i = p5w2.tile([128, JT, 128], F8E4, tag="w2i",
                                     name=f"w2i{i}")
                    nc.gpsimd.dma_start(w2_i[:], w2_r[:, :, i * 128:(i + 1) * 128])
                    w2_tiles.append(w2_i)
                with tc.tile_pool(name="p5_w", bufs=3) as p5w, \
                     tc.tile_pool(name="p5_work", bufs=3) as work5, \
                     tc.tile_pool(name="p5_psg", bufs=4, space="PSUM") as psg, \
                     tc.tile_pool(name="p5_psx", bufs=4, space="PSUM") as psx:
                    for j in range(JT):
                        w1g_j = p5w.tile([128, CT, 128], F8E4, tag="w1gj")
                        nc.sync.dma_start(w1g_j[:], w1g_r[:, :, j * 128:(j + 1) * 128])
                        w1x_j = p5w.tile([128, CT, 128], F8E4, tag="w1xj")
                        nc.sync.dma_start(w1x_j[:], w1x_r[:, :, j * 128:(j + 1) * 128])
                        g_ps = [psg.tile([128, 512], F32, tag="g",
                                         name=f"gps{t}") for t in range(2)]
                        x_ps = [psx.tile([128, 512], F32, tag="x",
                                         name=f"xps{t}") for t in range(2)]
                        for j2 in range(CT // 2):
                            for tcx in range(2):
                                sl = slice(tcx * 512, (tcx + 1) * 512)
                                nc.tensor.matmul(
                                    g_ps[tcx][:], w1g_j[:, 2 * j2:2 * j2 + 2, :],
                                    x2_f8[:, 2 * j2:2 * j2 + 2, sl],
                                    start=(j2 == 0), stop=(j2 == CT // 2 - 1),
                                    perf_mode=DRM)
                        for j2 in range(CT // 2):
                            for tcx in range(2):
                                sl = slice(tcx * 512, (tcx + 1) * 512)
                                nc.tensor.matmul(
                                    x_ps[tcx][:], w1x_j[:, 2 * j2:2 * j2 + 2, :],
                                    x2_f8[:, 2 * j2:2 * j2 + 2, sl],
                                    start=(j2 == 0), stop=(j2 == CT // 2 - 1),
                                    perf_mode=DRM)
                        for tcx in range(2):
                            g_sb = work5.tile([128, 512], BF16, tag="gsb")
                            nc.scalar.activation(out=g_sb[:], in_=g_ps[tcx][:],
                                                 func=AF.Silu, bias=b1g[:, j:j + 1])
                            nc.vector.scalar_tensor_tensor(
                                out=hp[:, j, tcx * 512:(tcx + 1) * 512],
                                in0=x_ps[tcx][:], scalar=b1x[:, j:j + 1],
                                in1=g_sb[:], op0=OP.add, op1=OP.mult)
                with tc.tile_pool(name="p5_work2", bufs=3) as work52, \
                     tc.tile_pool(name="p5_psf", bufs=3, space="PSUM") as psf:
                    for i in range(CT):
                        w2_i = w2_tiles[i]
                        for tcx in range(2):
                            sl = slice(tcx * 512, (tcx + 1) * 512)
                            f_ps = psf.tile([128, 512], F32, tag="f")
                            first = True
                            if b2r is not None:
                                nc.tensor.matmul(f_ps[:],
                                                 b2r[0:1, i * 128:(i + 1) * 128],
                                                 ones_row[:], start=True, stop=False)
                                first = False
                            for j2 in range(JT // 2):
                                nc.tensor.matmul(
                                    f_ps[:], w2_i[:, 2 * j2:2 * j2 + 2, :],
                                    hp[:, 2 * j2:2 * j2 + 2, sl],
                                    start=first and (j2 == 0),
                                    stop=(j2 == JT // 2 - 1),
                                    perf_mode=DRM)
                            y_sb = work52.tile([128, 512], F32, tag="ysb")
                            nc.vector.scalar_tensor_tensor(
                                out=y_sb[:], in0=f_ps[:], scalar=ls2[:, i:i + 1],
                                in1=x1_f32[:, i, sl], op0=OP.mult, op1=OP.add)
                            nc.gpsimd.dma_start(Y[i * 128:(i + 1) * 128, sl], y_sb[:])

    nc.compile()
    return nc


def _rope_tables(rope, g, b):
    """cos/W (swap-multiplier) tables with gamma and the rstd 1/8-fold
    (tables x8); plus additive beta table (or None, unscaled)."""
    sin, cos = rope[:, :HD], rope[:, HD:]
    W = np.empty_like(sin)
    W[:, 0::2] = sin[:, 1::2]
    W[:, 1::2] = -sin[:, 0::2]
    c1 = (cos * g[None, :] * 8.0).astype(np.float32)
    w1 = (W * g[None, :] * 8.0).astype(np.float32)
    bt = None
    if b is not None and np.any(b):
        bw = b[None, :] * W
        bwsw = np.empty_like(bw)
        bwsw[:, 0::2], bwsw[:, 1::2] = bw[:, 1::2], bw[:, 0::2]
        bt = (b[None, :] * cos + bwsw).astype(np.float32)
    return np.ascontiguousarray(c1), np.ascontiguousarray(w1), bt


def _center_heads(w):
    """Subtract per-64-wide-head-block column mean: x@w then has exactly
    zero mean per head, so LN's mean subtraction is a no-op (LN is
    shift-invariant, so this does not change the reference math)."""
    w = np.asarray(w, np.float32).copy()
    for h0 in range(0, w.shape[1], HD):
        blk = w[:, h0:h0 + HD]
        blk -= blk.mean(axis=1, keepdims=True)
    return w


def _f8(a):
    return np.clip(np.asarray(a, np.float32), -240, 240).astype(
        ml_dtypes.float8_e4m3)


def _prepare(inputs):
    """Host-side sharding: returns (flags, in_maps) for the 8 cores."""
    f32 = np.float32
    bf = ml_dtypes.bfloat16
    x = np.asarray(inputs["x"], f32)
    ctx = np.asarray(inputs["ctx"], f32)
    rope = np.asarray(inputs["rope"], f32)

    flags = {
        "bq_sa": bool(np.any(inputs["sa_qb"])),
        "bk_sa": bool(np.any(inputs["sa_kb"])),
        "bq_ca": bool(np.any(inputs["ca_qb"])),
        "cakb": bool(np.any(inputs["ca_kb"])),
        "sa_bo": bool(np.any(inputs["sa_bo"])),
        "ca_bo": bool(np.any(inputs["ca_bo"])),
        "b2": bool(np.any(inputs["b2"])),
    }

    # guard the fp8e5m2 softmax fast path: |s*SCALE| must stay under ~9.1
    gq = np.asarray(inputs["sa_qg"], f32)
    gk = np.asarray(inputs["sa_kg"], f32)
    bq = np.asarray(inputs["sa_qb"], f32)
    bk = np.asarray(inputs["sa_kb"], f32)
    bound = 1.07 * (8 * np.abs(gq).max() + 8 * np.abs(bq).max()) * \
        (8 * np.abs(gk).max() + 8 * np.abs(bk).max()) / 8.0
    assert bound < 10.4, f"qk-norm bound {bound} too large for fp8 softmax"

    def fm(v, nt):  # feature-major [128, nt]
        return np.ascontiguousarray(np.asarray(v, f32).reshape(nt, 128).T)

    wqkv = np.asarray(inputs["wqkv"], f32).copy()
    wqkv[:, :C] = _center_heads(wqkv[:, :C])
    wqkv[:, C:2 * C] = _center_heads(wqkv[:, C:2 * C])

    shared = {
        "wqkv": _f8(wqkv),
        "sa_wo": _f8(inputs["sa_wo"]),
        "ca_wq": _f8(_center_heads(inputs["ca_wq"])),
        "ca_wk": _f8(_center_heads(inputs["ca_wk"])),
        "ca_wv": _f8(inputs["ca_wv"]),
        "ca_wo": _f8(inputs["ca_wo"]),
        "w1g": _f8(inputs["w1g"]),
        "w1x": _f8(inputs["w1x"]),
        "w2": _f8(inputs["w2"]),
        "b1g_f": fm(inputs["b1g"], JT),
        "b1x_f": fm(inputs["b1x"], JT),
        "ls0_f": fm(inputs["ls0"], CT),
        "ls1_f": fm(inputs["ls1"], CT),
        "ls2_f": fm(inputs["ls2"], CT),
        "gate_f": fm(inputs["ca_gate"], CT),
        "cakg": np.ascontiguousarray(np.tile(
            np.asarray(inputs["ca_kg"], f32)[None, :] * 8.0,
            (128, 1))).astype(bf),
    }
    if flags["sa_bo"]:
        shared["sa_bo_row"] = np.asarray(inputs["sa_bo"], f32).reshape(1, C).astype(bf)
    if flags["ca_bo"]:
        shared["ca_bo_row"] = np.asarray(inputs["ca_bo"], f32).reshape(1, C).astype(bf)
    if flags["b2"]:
        shared["b2_row"] = np.asarray(inputs["b2"], f32).reshape(1, C).astype(bf)
    if flags["cakb"]:
        shared["cakb"] = np.ascontiguousarray(
            np.tile(np.asarray(inputs["ca_kb"], f32)[None, :], (128, 1)))

    cq_sa, wq_sa, bq_sa = _rope_tables(rope, np.asarray(inputs["sa_qg"], f32),
                                       np.asarray(inputs["sa_qb"], f32))
    ck_sa, wk_sa, bk_sa = _rope_tables(rope, np.asarray(inputs["sa_kg"], f32),
                                       np.asarray(inputs["sa_kb"], f32))
    cq_ca, wq_ca, bq_ca = _rope_tables(rope, np.asarray(inputs["ca_qg"], f32),
                                       np.asarray(inputs["ca_qb"], f32))

    in_maps = []
    for core in range(8):
        b, h = divmod(core, 2)
        own = slice(h * NQ, (h + 1) * NQ)
        oth = slice((1 - h) * NQ, (2 - h) * NQ)
        perm = np.r_[own, oth]
        xp = x[b][perm]                      # [2048, 1024] own rows first
        m = dict(shared)
        m["xT"] = _f8(np.ascontiguousarray(xp.T))
        m["x_own"] = np.ascontiguousarray(x[b][own].T)
        m["ctxT"] = _f8(np.ascontiguousarray(ctx[b].T))
        m["cosq_sa"] = cq_sa[own].astype(bf)
        m["wq_sa"] = wq_sa[own].astype(bf)
        m["cosk_sa"] = np.ascontiguousarray(ck_sa[perm]).astype(bf)
        m["wk_sa"] = np.ascontiguousarray(wk_sa[perm]).astype(bf)
        m["cosq_ca"] = cq_ca[own].astype(bf)
        m["wq_ca"] = wq_ca[own].astype(bf)
        if flags["bq_sa"]:
            m["bq_sa"] = bq_sa[own]
        if flags["bk_sa"]:
            m["bk_sa"] = np.ascontiguousarray(bk_sa[perm])
        if flags["bq_ca"]:
            m["bq_ca"] = bq_ca[own]
        in_maps.append(m)
    return flags, in_maps


def _get_program(flags):
    key = tuple(sorted(flags.items()))
    if key not in _CACHE:
        _CACHE[key] = _build_program(flags)
    return _CACHE[key]


def _run(in_maps, nc, trace=False, trace_kwargs=None):
    return run_bass_kernel_spmd(nc, in_maps, list(range(8)), trace=trace,
                                **(trace_kwargs or {}))


def kernel(**inputs):
    flags, in_maps = _prepare(inputs)
    nc = _get_program(flags)
    res = _run(in_maps, nc)
    out = np.empty((B, N, C), np.float32)
    for core in range(8):
        b, h = divmod(core, 2)
        out[b, h * NQ:(h + 1) * NQ, :] = res.results[core]["y"].T
    return out
